# revision 1
# baseline (speedup 1.0000x reference)
"""MoE routing kernel for Trainium2 (8 NeuronCores, Bass/Tile).

Strategy (expert-parallel, two SPMD launches):
  Phase A  - tokens sharded 128/core. Each core computes the gate MLP
             (d->4d->4d->E, gelu/gelu/sigmoid) in fp32, then top-2 +
             normalization on device (nc.vector.max top-8 op), emitting
             a sparse weight matrix w[128, 64] (2 nonzeros per row).
  Host     - pure routing/layout: group token ids by expert id, gather
             token activations per expert (transposed), pad to capacity.
  Phase B  - experts sharded 8/core. Each core streams its 8 experts'
             pre-transposed fp16 weights from HBM once (the memory-bound
             term), runs the 2-layer FFN for the tokens routed to each
             expert (fp16 matmuls, fp32 PSUM accumulate), applies gelu
             and the gate weight on device.
  Host     - unshard: scatter-add the disjoint per-expert rows back to
             token order.

Precision: the gate must stay true fp32 - the min rank2/rank3 gate gap
for this model is ~2e-6, so bf16/f32r matmul noise flips routing for ~1%
of tokens (verified on HW: f32r gave 0.36 rel err). The expert FFN runs
in fp16 (10-bit mantissa, fp32 accumulate): measured 4.0e-4 absmax-rel
error end-to-end vs the fp32 reference (bf16 would be 2.9e-3, fp32
5e-7 but ~5x slower on PE: fp32 matmul = 2-pass LOW_HIGH + 333ns
4-byte LDWEIGHTS per pass; fp16 gets single pass + fast weight load).

Measured on HW (8 cores, NTFF profile): phase A ~27us + phase B ~28us.
~17us of each launch is fixed NEFF overhead (engine rendezvous, ACT
table loads, tail drain/barrier).
"""

import os
import sys

sys.path.insert(0, "/opt/trn_rl_repo")

# The kernel executes through the axon PJRT proxy; a CPU pin (e.g. from a
# harness that runs the jax reference on CPU) would break device dispatch.
# Only effective if jax hasn't been imported yet in this process.
if os.environ.get("JAX_PLATFORMS") == "cpu" and "jax" not in sys.modules:
    del os.environ["JAX_PLATFORMS"]

import numpy as np

import concourse.bass as bass
import concourse.tile as tile
from concourse import bacc, mybir
from concourse.bass_utils import run_bass_kernel_spmd

F32 = mybir.dt.float32
F32R = mybir.dt.float32r
AFT = mybir.ActivationFunctionType
ALU = mybir.AluOpType

N_CORES = 8
DIM = 128          # model dim d
HID = 512          # expert / gate hidden = 4d
NEXP = 64          # experts
SEQ = 1024         # tokens
TPC = SEQ // N_CORES    # tokens per core (phase A) = 128
ELOC = NEXP // N_CORES  # experts per core (phase B) = 8
KC = HID // 128         # 4 contraction chunks of 128 over the hidden dim

# matmul operand dtype: "f32" (exact, 2-pass) or "f32r" (single-pass)
MM_DT = os.environ.get("MOE_MM_DT", "f32")
# phase A sharding: "tok128" (8 cores x 128 tokens) or "split256"
# (4 unique slices x 256 tokens, cores 4-7 duplicate; N=256 makes
# f32r matmuls single-cycle-per-row)
A_MODE = os.environ.get("MOE_A_MODE", "tok128")
# phase B matmul dtype: "" (= MM_DT path), "bf16", or "f16"
B_DT = os.environ.get("MOE_B_DT", "f16")
BF16 = mybir.dt.bfloat16
FP16 = mybir.dt.float16

# phase A packed-constants tile [128, ABLOB_W]:
#   [0:4)   gb1h (bias per-partition, col mc)   [4:8) gb2h
#   partition 0, cols [16:80) = gb3 row
ABLOB_W = 80
# phase B packed-constants tile [128, BBLOB_W]:
#   [0:32)  b1h (col j*4+kc)    [32:40) wv (slot=partition, col j)
BBLOB_W = 40

last_run_info = {}


def _mm_ap(ap):
    return ap


def _ensure_axon_ntff_hook():
    """Provide antenv.axon_hooks (NTFF profiling hook) if the image lacks it."""
    try:
        import antenv.axon_hooks  # noqa: F401

        return
    except ImportError:
        pass
    import contextlib
    import ctypes
    import types

    mod = types.ModuleType("antenv.axon_hooks")
    holder = {"h": None}
    mod.set_axon_ntff_profile_hook = lambda h: holder.__setitem__("h", h)
    mod.get_axon_ntff_profile_hook = lambda: holder["h"]
    sys.modules["antenv.axon_hooks"] = mod
    try:
        import antenv

        antenv.axon_hooks = mod
    except ImportError:
        pass

    so_path = "/opt/axon/libaxon_pjrt.so"
    if not os.path.exists(so_path):
        return
    try:
        lib = ctypes.CDLL(so_path)
        if not hasattr(lib, "axon_start_nrt_profile"):
            return
        lib.axon_start_nrt_profile.argtypes = [
            ctypes.POINTER(ctypes.c_int64),
            ctypes.c_size_t,
        ]
        lib.axon_start_nrt_profile.restype = ctypes.c_int64
        lib.axon_stop_nrt_profile.argtypes = [ctypes.c_char_p]
        lib.axon_stop_nrt_profile.restype = ctypes.c_int64

        @contextlib.contextmanager
        def _hook(output_dir, device_ids):
            import jax

            jax.devices()
            if device_ids:
                ids = (ctypes.c_int64 * len(device_ids))(*device_ids)
                rc = lib.axon_start_nrt_profile(ids, len(device_ids))
            else:
                rc = lib.axon_start_nrt_profile(None, 0)
            if rc != 0:
                raise RuntimeError(f"axon_start_nrt_profile rc={rc}")
            try:
                yield
            finally:
                n = lib.axon_stop_nrt_profile(str(output_dir).encode())
                print(f"profile: {n} file(s) -> {output_dir}", file=sys.stderr)

        mod.set_axon_ntff_profile_hook(_hook)
    except Exception:
        pass


def _build_phase_a(has_gb1, has_gb2, has_gb3, act=None, tpc=TPC):
    """Gate MLP + top2 for tpc tokens. SPMD over 8 cores."""
    act = act or AFT.Gelu
    adt = F32R if MM_DT == "f32r" else F32
    nc = bacc.Bacc(
        "TRN2", target_bir_lowering=False, debug=False, num_devices=N_CORES
    )
    xT = nc.declare_dram_parameter("xT", [DIM, tpc], adt, isOutput=False)
    gw1 = nc.declare_dram_parameter("gw1", [DIM, HID], adt, isOutput=False)
    gw2p = nc.declare_dram_parameter("gw2p", [128, KC * HID], adt, isOutput=False)
    gw3p = nc.declare_dram_parameter("gw3p", [128, KC * NEXP], adt, isOutput=False)
    ablob = nc.declare_dram_parameter("ablob", [128, ABLOB_W], F32, isOutput=False)
    wout = nc.declare_dram_parameter("wout", [tpc, NEXP], F32, isOutput=True)

    with tile.TileContext(nc) as tc:
        with (
            tc.tile_pool(name="sb", bufs=1) as sb,
            tc.tile_pool(name="ps", bufs=1, space="PSUM") as ps,
        ):
            # split input loads across both HWDGE issue engines (sync +
            # scalar) - issue cost is ~0.65us per dma_start per engine
            gw1_t = sb.tile([DIM, HID], adt, tag="gw1")
            nc.sync.dma_start(gw1_t[:], gw1.ap())
            xT_t = sb.tile([DIM, tpc], adt, tag="xT")
            nc.scalar.dma_start(xT_t[:], xT.ap())
            gw3_t = sb.tile([128, KC * NEXP], adt, tag="gw3")
            nc.scalar.dma_start(gw3_t[:], gw3p.ap())
            ab_t = sb.tile([128, ABLOB_W], F32, tag="ablob")
            nc.scalar.dma_start(ab_t[:], ablob.ap())
            gw2_t = sb.tile([128, KC * HID], adt, tag="gw2")
            for kc in range(KC):
                nc.sync.dma_start(
                    gw2_t[:, kc * HID : (kc + 1) * HID],
                    gw2p.ap()[:, kc * HID : (kc + 1) * HID],
                )
            ones_t = sb.tile([1, tpc], F32, tag="ones")
            nc.vector.memset(ones_t[:], 1.0)

            # H1T[f, t] = gelu(gw1.T @ xT + b1), feature-major, 4 chunks
            h1 = sb.tile([128, KC * tpc], adt, tag="h1")
            for mc in range(KC):
                p = ps.tile([128, tpc], F32, tag="h1ps", bufs=2)
                nc.tensor.matmul(
                    p[:],
                    _mm_ap(gw1_t[:, mc * 128 : (mc + 1) * 128]),
                    _mm_ap(xT_t[:]),
                    start=True,
                    stop=True,
                )
                nc.scalar.activation(
                    h1[:, mc * tpc : (mc + 1) * tpc],
                    p[:],
                    act,
                    bias=ab_t[:, mc : mc + 1] if has_gb1 else 0.0,
                )

            # H2T[f, t] = gelu(gw2.T @ H1T + b2)
            h2 = sb.tile([128, KC * tpc], adt, tag="h2")
            for mc in range(KC):
                p = ps.tile([128, tpc], F32, tag="h2ps", bufs=3)
                for kc in range(KC):
                    nc.tensor.matmul(
                        p[:],
                        _mm_ap(gw2_t[:, kc * HID + mc * 128 : kc * HID + (mc + 1) * 128]),
                        _mm_ap(h1[:, kc * tpc : (kc + 1) * tpc]),
                        start=(kc == 0),
                        stop=(kc == KC - 1),
                    )
                nc.scalar.activation(
                    h2[:, mc * tpc : (mc + 1) * tpc],
                    p[:],
                    act,
                    bias=ab_t[:, 4 + mc : 5 + mc] if has_gb2 else 0.0,
                )

            # G[t, e] = sigmoid(H2 @ gw3 + b3), token-major, per
            # 128-token group
            for tg in range(tpc // 128):
                gp = ps.tile([128, NEXP], F32, tag="gps", bufs=2)
                for kc in range(KC):
                    nc.tensor.matmul(
                        gp[:],
                        _mm_ap(h2[:, kc * tpc + tg * 128 : kc * tpc + (tg + 1) * 128]),
                        _mm_ap(gw3_t[:, kc * NEXP : (kc + 1) * NEXP]),
                        start=(kc == 0),
                        stop=(kc == KC - 1) and not has_gb3,
                    )
                if has_gb3:
                    nc.tensor.matmul(
                        gp[:],
                        ones_t[0:1, 0:128],
                        ab_t[0:1, 16 : 16 + NEXP],
                        start=False,
                        stop=True,
                    )
                g = sb.tile([128, NEXP], F32, tag=f"g{tg}")
                nc.scalar.activation(g[:], gp[:], AFT.Sigmoid)

                # top-2 -> normalized sparse weights
                mx = sb.tile([128, 8], F32, tag=f"mx{tg}")
                nc.vector.max(mx[:], g[:])
                ssum = sb.tile([128, 1], F32, tag=f"ssum{tg}")
                nc.vector.tensor_add(ssum[:], mx[:, 0:1], mx[:, 1:2])
                rcp = sb.tile([128, 1], F32, tag=f"rcp{tg}")
                nc.vector.reciprocal(rcp[:], ssum[:])
                mask = sb.tile([128, NEXP], F32, tag=f"mask{tg}")
                nc.vector.tensor_scalar(
                    mask[:], g[:], mx[:, 1:2], None, op0=ALU.is_ge
                )
                wts = sb.tile([128, NEXP], F32, tag=f"wts{tg}")
                nc.vector.tensor_mul(wts[:], g[:], mask[:])
                wts2 = sb.tile([128, NEXP], F32, tag=f"wts2{tg}")
                nc.vector.tensor_scalar_mul(wts2[:], wts[:], rcp[:])
                nc.sync.dma_start(
                    wout.ap()[tg * 128 : (tg + 1) * 128, :], wts2[:]
                )
    nc.compile()
    return nc


def _build_phase_b(cap, has_b1, has_b2, act=None):
    """Expert FFN for ELOC experts x cap token slots. SPMD over 8 cores."""
    act = act or AFT.Gelu
    if B_DT == "bf16":
        mdt = BF16
    elif B_DT == "f16":
        mdt = FP16
    elif MM_DT == "f32r":
        mdt = F32R
    else:
        mdt = F32
    mm = lambda ap: ap
    nc = bacc.Bacc(
        "TRN2", target_bir_lowering=False, debug=False, num_devices=N_CORES
    )
    wt = nc.declare_dram_parameter("wt", [ELOC, 128, 1024], mdt, isOutput=False)
    xe = nc.declare_dram_parameter("xe", [DIM, ELOC * cap], mdt, isOutput=False)
    blob = nc.declare_dram_parameter("blob", [128, BBLOB_W], F32, isOutput=False)
    rdt = mdt if B_DT in ("bf16", "f16") else F32
    b2r = nc.declare_dram_parameter("b2r", [1, ELOC * DIM], rdt, isOutput=False)
    yout = nc.declare_dram_parameter("yout", [cap, ELOC * DIM], F32, isOutput=True)

    with tile.TileContext(nc) as tc:
        with (
            tc.tile_pool(name="sb", bufs=1) as sb,
            tc.tile_pool(name="wtp", bufs=3) as wtp,
            tc.tile_pool(name="tp", bufs=2) as tp,
            tc.tile_pool(name="yp", bufs=2) as yp,
            tc.tile_pool(name="psT", bufs=2, space="PSUM") as psT,
            tc.tile_pool(name="psY", bufs=2, space="PSUM") as psY,
        ):
            # expert 0's W1 half goes first so the first matmul can
            # start as early as possible; the rest of its weights follow
            # after the (small) activation/constant loads.
            wt0_t = wtp.tile([128, 1024], mdt, tag="wt0")
            nc.sync.dma_start(wt0_t[:, 0:512], wt.ap()[0][:, 0:512])
            xe_t = sb.tile([DIM, ELOC * cap], mdt, tag="xe")
            nc.scalar.dma_start(xe_t[:], xe.ap())
            blob_t = sb.tile([128, BBLOB_W], F32, tag="blob")
            nc.scalar.dma_start(blob_t[:], blob.ap())
            nc.sync.dma_start(wt0_t[:, 512:1024], wt.ap()[0][:, 512:1024])
            if has_b2:
                b2r_t = sb.tile([1, ELOC * DIM], rdt, tag="b2r")
                nc.sync.dma_start(b2r_t[:], b2r.ap())
                ones_t = sb.tile([1, cap], rdt, tag="ones")
                nc.vector.memset(ones_t[:], 1.0)
            out_sb = sb.tile([cap, ELOC * DIM], F32, tag="out")

            wt_tiles = [wt0_t]
            for j in range(1, ELOC):
                wt_t = wtp.tile([128, 1024], mdt, tag=f"wt{j}")
                eng = nc.scalar if j % 2 == 0 else nc.sync
                eng.dma_start(wt_t[:], wt.ap()[j])
                wt_tiles.append(wt_t)

            for pr in range(ELOC // 2):
                js = (2 * pr, 2 * pr + 1)
                # T[f, slot] = gelu(W1 @ xe_j + B1) for both experts of the
                # pair into one psum tile; single fused gelu when B1 == 0.
                pT = psT.tile([128, 1024], F32, tag="pT")
                for jj, j in enumerate(js):
                    for kc in range(KC):
                        nc.tensor.matmul(
                            pT[:, jj * 512 + kc * cap : jj * 512 + (kc + 1) * cap],
                            mm(wt_tiles[j][:, kc * 128 : (kc + 1) * 128]),
                            mm(xe_t[:, j * cap : (j + 1) * cap]),
                            start=True,
                            stop=True,
                        )
                t_sb = tp.tile([128, 1024], mdt, tag="t")
                if has_b1:
                    for jj, j in enumerate(js):
                        for kc in range(KC):
                            nc.scalar.activation(
                                t_sb[:, jj * 512 + kc * cap : jj * 512 + (kc + 1) * cap],
                                pT[:, jj * 512 + kc * cap : jj * 512 + (kc + 1) * cap],
                                act,
                                bias=blob_t[:, j * KC + kc : j * KC + kc + 1],
                            )
                else:
                    nc.scalar.activation(t_sb[:], pT[:], act)

                # Y[slot, d] = gelu(T.T @ W2.T + B2), token-major, pair-fused
                pY = psY.tile([cap, 2 * DIM], F32, tag="pY")
                for jj, j in enumerate(js):
                    for kc in range(KC):
                        nc.tensor.matmul(
                            pY[:, jj * DIM : (jj + 1) * DIM],
                            mm(t_sb[:, jj * 512 + kc * cap : jj * 512 + (kc + 1) * cap]),
                            mm(wt_tiles[j][:, 512 + kc * 128 : 512 + (kc + 1) * 128]),
                            start=(kc == 0),
                            stop=(kc == KC - 1) and not has_b2,
                        )
                    if has_b2:
                        nc.tensor.matmul(
                            pY[:, jj * DIM : (jj + 1) * DIM],
                            mm(ones_t[0:1, :]),
                            mm(b2r_t[0:1, j * DIM : (j + 1) * DIM]),
                            start=False,
                            stop=True,
                        )
                y_sb = yp.tile([cap, 2 * DIM], F32, tag="y")
                nc.scalar.activation(y_sb[:], pY[:], act)
                for jj, j in enumerate(js):
                    # scale by gate weight (per-partition = per-slot scalar)
                    nc.vector.tensor_scalar_mul(
                        out_sb[:, j * DIM : (j + 1) * DIM],
                        y_sb[:, jj * DIM : (jj + 1) * DIM],
                        blob_t[0:cap, 32 + j : 33 + j],
                    )
                # stream this pair's rows out while later pairs compute
                nc.sync.dma_start(
                    yout.ap()[:, pr * 2 * DIM : (pr + 1) * 2 * DIM],
                    out_sb[:, pr * 2 * DIM : (pr + 1) * 2 * DIM],
                )
    nc.compile()
    return nc


def _run(nc, in_maps, label):
    trace = bool(os.environ.get("BASS_TRACE"))
    kwargs = {}
    if trace:
        _ensure_axon_ntff_hook()
        tmpdir = os.path.join("/tmp", f"moe_{label}")
        os.makedirs(tmpdir, exist_ok=True)
        kwargs["tmpdir"] = tmpdir
    res = run_bass_kernel_spmd(
        nc, in_maps, core_ids=list(range(N_CORES)), trace=trace, **kwargs
    )
    last_run_info[label] = {
        "exec_time_ns": res.exec_time_ns,
        "mean_exec_time_ns": res.mean_exec_time_ns,
        "trace": (res.instructions_and_trace or (None, None))[1],
    }
    return res.results


def kernel(x, gw1, gb1, gw2, gb2, gw3, gb3, W1, B1, W2, B2):
    x = np.ascontiguousarray(np.asarray(x, np.float32))
    xf = x.reshape(SEQ, DIM)
    gb1 = np.asarray(gb1, np.float32)
    gb2 = np.asarray(gb2, np.float32)
    gb3 = np.asarray(gb3, np.float32)
    has_gb1 = bool(np.any(gb1))
    has_gb2 = bool(np.any(gb2))
    has_gb3 = bool(np.any(gb3))

    # ---------------- Phase A: gate + top2 ----------------
    tpc_a = 256 if A_MODE == "split256" else TPC
    ncA = _build_phase_a(has_gb1, has_gb2, has_gb3, tpc=tpc_a)
    gw2np = np.asarray(gw2, np.float32)
    gw3np = np.asarray(gw3, np.float32)
    gw2p = np.ascontiguousarray(
        gw2np.reshape(KC, 128, HID).transpose(1, 0, 2).reshape(128, KC * HID)
    )
    gw3p = np.ascontiguousarray(
        gw3np.reshape(KC, 128, NEXP).transpose(1, 0, 2).reshape(128, KC * NEXP)
    )
    ablob = np.zeros((128, ABLOB_W), np.float32)
    ablob[:, 0:KC] = gb1.reshape(KC, 128).T
    ablob[:, KC : 2 * KC] = gb2.reshape(KC, 128).T
    ablob[0, 16 : 16 + NEXP] = gb3
    gw1c = np.ascontiguousarray(gw1, np.float32)
    n_slices = SEQ // tpc_a
    in_maps_a = []
    for c in range(N_CORES):
        sl = c % n_slices
        xs = xf[sl * tpc_a : (sl + 1) * tpc_a]
        in_maps_a.append(
            dict(
                xT=np.ascontiguousarray(xs.T),
                gw1=gw1c,
                gw2p=gw2p,
                gw3p=gw3p,
                ablob=ablob,
            )
        )
    res_a = _run(ncA, in_maps_a, "phase_a")
    w = np.concatenate([res_a[c]["wout"] for c in range(n_slices)], axis=0)

    # ---------------- Host routing (indexing only) ----------------
    toks = [np.nonzero(w[:, e])[0] for e in range(NEXP)]
    max_n = max(len(t) for t in toks)
    cap = max(32, -(-max_n // 32) * 32)
    assert cap <= 128, f"per-expert capacity {cap} exceeds one partition tile"

    W1 = np.asarray(W1, np.float32)
    W2 = np.asarray(W2, np.float32)
    B1 = np.asarray(B1, np.float32)
    B2 = np.asarray(B2, np.float32)
    has_b1 = bool(np.any(B1))
    has_b2 = bool(np.any(B2))

    import ml_dtypes

    bdt = {"bf16": ml_dtypes.bfloat16, "f16": np.float16}.get(B_DT, np.float32)
    in_maps_b = []
    for c in range(N_CORES):
        wt = np.zeros((ELOC, 128, 1024), bdt)
        xe = np.zeros((DIM, ELOC * cap), bdt)
        blob = np.zeros((128, BBLOB_W), np.float32)
        b2r = np.zeros((1, ELOC * DIM), bdt if B_DT in ("bf16", "f16") else np.float32)
        for j in range(ELOC):
            e = c * ELOC + j
            wt[j, :, 0:512] = W1[e].T
            wt[j, :, 512:1024] = (
                W2[e].reshape(128, KC, 128).transpose(2, 1, 0).reshape(128, 512)
            )
            blob[:, j * KC : (j + 1) * KC] = B1[e].reshape(KC, 128).T
            b2r[0, j * DIM : (j + 1) * DIM] = B2[e]
            te = toks[e]
            xe[:, j * cap : j * cap + len(te)] = xf[te].T
            blob[: len(te), 32 + j] = w[te, e]
        in_maps_b.append(dict(wt=wt, xe=xe, blob=blob, b2r=b2r))

    ncB = _build_phase_b(cap, has_b1, has_b2)
    res_b = _run(ncB, in_maps_b, "phase_b")

    # ---------------- Host unshard: scatter-add ----------------
    y = np.zeros((SEQ, DIM), np.float32)
    for c in range(N_CORES):
        yo = res_b[c]["yout"]  # [cap, ELOC*DIM]
        for j in range(ELOC):
            e = c * ELOC + j
            te = toks[e]
            y[te] += yo[: len(te), j * DIM : (j + 1) * DIM]
    return y.reshape(1, SEQ, DIM)

